# revision 9
# baseline (speedup 1.0000x reference)
"""Multi-head attention (B=2, L=2048, D=1024, H=16, DK=64) on 8 TRN2 NeuronCores.

Sharding: core c handles batch b = c//4 and head-group g = c%4 (4 heads,
256 model dims). Per-core compute (no collectives):
  QT/KT  [256, 2048] projections in [dk, seq] layout (rhs = x^T, lhsT = w^T)
  V      [2048, 260] with a fused ones-column per head (softmax Z for free)
  S^T    = K_h @ Q_h^T per head in [keys, queries] layout (row-packed head
           pairs run CONCURRENT on the PE array, K=64 each)
  P      = exp(S^T / 8)     one ACT op per (key-tile, head-pair)
  ctx^T  = V'_h^T @ P       -> rows 0:64 ctx, row 64 = Z
  ctx   /= Z                (reciprocal + gpsimd partition broadcast + mul)
  out_g  = ctx_g @ w_o[:, g]^T   -> per-core PARTIAL output [2048, 1024]
Host sums the 4 head-group partials per batch and stacks the 2 batches.

Schedule: ACT(exp)-centric. The scalar engine is the roofline (128 exps of
FD=1024 at (1024+352)/1.2 ns each ~= 147us); everything else (projections,
out-proj, DMA) is filler emitted around the exp stream. Unit order
(0,0),(0,1),(1,0),(1,1),(0,2),(1,2) then the last query chunk split in two
256-wide subchunks to shrink the tail. Inputs land as contiguous pre-tiled
blocks on both HW DMA queues + gpsimd SWDGE; outputs alternate HW queues.
"""

import numpy as np

D = 1024
L = 2048
DK = 64
GH = 4           # heads per core
GD = GH * DK     # model dims per core (256)
NCORES = 8
ND = D // 128    # 8 d-tiles
NL = L // 128    # 16 key tiles


def _build():
    import concourse.bacc as bacc
    import concourse.mybir as mybir
    import concourse.tile as tile
    from concourse.tile import add_dep_helper

    f32 = mybir.dt.float32
    bf16 = mybir.dt.bfloat16
    Exp = mybir.ActivationFunctionType.Exp

    nc = bacc.Bacc("TRN2", target_bir_lowering=False, debug=False,
                   num_devices=NCORES)
    xqd = nc.dram_tensor("xq", [128, 4 * ND * 512], bf16,
                         kind="ExternalInput").ap()
    wqd = nc.dram_tensor("wqT", [128, ND * GD], bf16, kind="ExternalInput").ap()
    wkd = nc.dram_tensor("wkT", [128, ND * GD], bf16, kind="ExternalInput").ap()
    wvd = nc.dram_tensor("wvT", [128, ND * GD], bf16, kind="ExternalInput").ap()
    wod = nc.dram_tensor("woT", [128, 2 * D], bf16, kind="ExternalInput").ap()
    out = nc.dram_tensor("out", [L, D], f32, kind="ExternalOutput").ap()

    with tile.TileContext(nc) as tc:
        with (
            tc.tile_pool(name="xp", bufs=1) as xp,
            tc.tile_pool(name="wp", bufs=1) as wp,
            tc.tile_pool(name="qk", bufs=1) as qk,
            tc.tile_pool(name="vp", bufs=1) as vp,
            tc.tile_pool(name="cx", bufs=1) as cx,
            tc.tile_pool(name="pp", bufs=9) as pp,
            tc.tile_pool(name="rp", bufs=4) as rp,
            tc.tile_pool(name="op", bufs=3) as op_,
            tc.tile_pool(name="ps", bufs=2, space="PSUM") as ps,
            tc.tile_pool(name="pc", bufs=2, space="PSUM") as pc,
            tc.tile_pool(name="po", bufs=2, space="PSUM") as pop,
        ):
            # ---- engine warmup (runs during the DMA wait) -------------------
            wsb = wp.tile([128, 512], bf16, tag="wsb", name="wsb")
            nc.vector.memset(wsb[:], 0.0)
            wact = wp.tile([128, 128], bf16, tag="wact", name="wact")
            # triggers the exp ACT_TABLE_LOAD early
            nc.scalar.activation(wact[:], wsb[:, 0:128], Exp, scale=0.125)
            wpo = pop.tile([128, 512], f32, tag="o", name="o")
            for _ in range(8):   # keep PE busy so HAM un-throttles early
                nc.tensor.matmul(wpo[:], wsb[:, 0:128], wsb[:],
                                 start=True, stop=True)

            # ---- DMA inputs -------------------------------------------------
            wkts = wp.tile([128, ND, GD], bf16, tag="wk", name="wk")
            wqts = wp.tile([128, ND, GD], bf16, tag="wq", name="wq")
            wvts = wp.tile([128, ND, GD], bf16, tag="wv", name="wv")
            wots = wp.tile([128, 2, D], bf16, tag="wo", name="wo")
            xta = xp.tile([128, 4, ND, 512], bf16, tag="x", name="x")
            # sync queue: wk, x q0, x q2;  scalar queue: wv, x q1, x q3
            # gpsimd swdge: wq, wo
            nc.sync.dma_start(wkts[:], wkd.rearrange("p (d c) -> p d c", d=ND))
            nc.scalar.dma_start(wvts[:], wvd.rearrange("p (d c) -> p d c", d=ND))
            xr = xqd.rearrange("p (q d c) -> p q d c", q=4, d=ND)
            nc.sync.dma_start(xta[:, 0], xr[:, 0])
            nc.scalar.dma_start(xta[:, 1], xr[:, 1])
            nc.sync.dma_start(xta[:, 2], xr[:, 2])
            nc.scalar.dma_start(xta[:, 3], xr[:, 3])
            nc.gpsimd.dma_start(wqts[:], wqd.rearrange("p (d c) -> p d c", d=ND))
            nc.gpsimd.dma_start(wots[:], wod.rearrange("p (i c) -> p i c", i=2))

            # ---- persistent SBUF tensors -----------------------------------
            # V with ones column per head: [keys 128, key-tile, DK+1]
            vph = [vp.tile([128, NL, DK + 1], bf16, tag=f"v{h}", name=f"v{h}")
                   for h in range(GH)]
            onesc = wp.tile([128, NL, 1], f32, tag="ones", name="ones")
            nc.vector.memset(onesc[:], 1.0)
            for h in range(GH):
                nc.vector.tensor_copy(vph[h][:, :, DK:DK + 1], onesc[:])
            qth = [qk.tile([128, L], bf16, tag=f"q{hp}", name=f"q{hp}")
                   for hp in range(2)]
            kth = [qk.tile([128, L], bf16, tag=f"k{hp}", name=f"k{hp}")
                   for hp in range(2)]
            ctxt = [cx.tile([128, L], bf16, tag=f"c{hp}", name=f"c{hp}")
                    for hp in range(2)]

            # ---- helpers ----------------------------------------------------
            def proj_qk_chunk(hp, w_sb, dstl, qc):
                acc = pop.tile([128, 512], f32, tag="o", name="o")
                for d in range(ND):
                    nc.tensor.matmul(
                        acc[:], w_sb[:, d, hp * 128:(hp + 1) * 128],
                        xta[:, qc, d, :],
                        start=(d == 0), stop=(d == ND - 1))
                nc.vector.tensor_copy(
                    dstl[hp][:, qc * 512:(qc + 1) * 512], acc[:])

            def v_proj_tile(lt):
                q, r = lt // 4, lt % 4
                acc = pop.tile([128, 512], f32, tag="o", name="o")
                for d in range(ND):
                    nc.tensor.matmul(
                        acc[:, 0:GD],
                        xta[:, q, d, r * 128:(r + 1) * 128],
                        wvts[:, d, :],
                        start=(d == 0), stop=(d == ND - 1))
                for h in range(GH):
                    nc.vector.tensor_copy(
                        vph[h][:, lt, 0:DK],
                        acc[:, h * DK:(h + 1) * DK])

            def attn(hp, q0, qw, fillers=None):
                """Attention for head-pair hp over queries [q0, q0+qw).

                fillers: dict lt -> list of callables, emitted after the lt's
                ctx matmuls (fills PE while ACT chews). Ends with the
                Z-division. Returns the per-lt tail ctx matmuls for pins.

                For qw=256 the two heads' score outputs go to separate PSUM
                banks (column offsets 0 and 512 of the s tile) so the
                row-packed matmul pair never shares a bank write port.
                """
                qsl = slice(q0, q0 + qw)
                cps = [pc.tile([DK + 1, 512], f32, tag="c", name=f"c{i}")
                       for i in range(2)]
                ctx_mms = []
                fd = 512 + qw   # activation covers [0 : 512+qw) of the s tile
                for lt in range(NL):
                    lsl = slice(lt * 128, (lt + 1) * 128)
                    sp = ps.tile([128, 1024], f32, tag="s", name="s")
                    for i in range(2):
                        row = slice(i * 64, (i + 1) * 64)
                        nc.tensor.matmul(
                            sp[:, i * 512:i * 512 + qw],
                            kth[hp][row, lsl], qth[hp][row, qsl],
                            start=True, stop=True,
                            tile_position=(i * 64, 0))
                    p = pp.tile([128, 1024], bf16, tag="p", name="p")
                    nc.scalar.activation(p[:, 0:fd], sp[:, 0:fd],
                                         Exp, scale=0.125)
                    for i in range(2):
                        mm = nc.tensor.matmul(
                            cps[i][:, 0:qw], vph[2 * hp + i][:, lt, :],
                            p[:, i * 512:i * 512 + qw],
                            start=(lt == 0), stop=(lt == NL - 1))
                        if i == 1:
                            ctx_mms.append(mm)
                    if fillers is not None and lt in fillers:
                        for f in fillers[lt]:
                            f()
                # division: evacuate ctx + Z, reciprocal, broadcast, multiply
                cus, rzs = [], []
                for i in range(2):
                    cu = pp.tile([64, 512], bf16, tag=f"cu{i}",
                                 name=f"cu{i}", bufs=2)
                    nc.vector.tensor_copy(cu[:, 0:qw], cps[i][0:DK, 0:qw])
                    zi = rp.tile([1, 512], f32, tag="zi", name="zi")
                    nc.vector.tensor_copy(zi[:, 0:qw], cps[i][DK:DK + 1, 0:qw])
                    cus.append(cu)
                    rzs.append(zi)
                for i in range(2):
                    rz = rp.tile([1, 512], f32, tag="rz", name="rz")
                    nc.vector.reciprocal_approx_fast(rz[:, 0:qw],
                                                     rzs[i][:, 0:qw])
                    rzs[i] = rz
                for i in range(2):
                    rzb = rp.tile([64, 512], f32, tag="rzb", name="rzb")
                    nc.gpsimd.partition_broadcast(rzb[:, 0:qw], rzs[i][:, 0:qw])
                    nc.vector.tensor_mul(
                        ctxt[hp][i * 64:(i + 1) * 64, qsl],
                        cus[i][:, 0:qw], rzb[:, 0:qw])
                return ctx_mms

            OUT_ENG = [nc.sync, nc.scalar]

            def out_proj(rows128, after=None, unit0=0):
                """Full output projection for the given 128-query row tiles."""
                unit = unit0
                for qt, rows in enumerate(rows128):
                    ot = op_.tile([128, 1024], f32, tag="ot", name="ot")
                    for ec in range(2):
                        esl = slice(ec * 512, (ec + 1) * 512)
                        po = pop.tile([128, 512], f32, tag="o", name="o")
                        for hp in range(2):
                            mm = nc.tensor.matmul(
                                po[:], ctxt[hp][:, rows],
                                wots[:, hp, esl],
                                start=(hp == 0), stop=(hp == 1))
                            if hp == 0 and after is not None:
                                pin = after[min(2 * unit + 1, len(after) - 1)]
                                add_dep_helper(mm.ins, pin.ins, sync=False,
                                               reason="pipeline out_proj")
                        unit += 1
                        nc.vector.tensor_copy(ot[:, esl], po[:])
                    OUT_ENG[qt % 2].dma_start(out[rows, :], ot[:])

            def rows_of(q0, qw):
                return [slice(q0 + qt * 128, q0 + (qt + 1) * 128)
                        for qt in range(qw // 128)]

            # ---- schedule ---------------------------------------------------
            proj_qk_chunk(0, wkts, kth, 0)
            proj_qk_chunk(0, wqts, qth, 0)
            v_proj_tile(0)

            K = proj_qk_chunk
            V = v_proj_tile
            # V(j) must be emitted at a filler slot <= j-1 (before its ctx
            # matmul); the PE over-commit here drains via the deep p pool.
            f00 = {
                0: [lambda: K(0, wkts, kth, 1), lambda: V(1)],
                1: [lambda: V(2)],
                2: [lambda: K(0, wkts, kth, 2), lambda: V(3)],
                3: [lambda: V(4)],
                4: [lambda: K(0, wkts, kth, 3), lambda: V(5)],
                5: [lambda: V(6)],
                6: [lambda: K(0, wqts, qth, 1), lambda: V(7)],
                7: [lambda: V(8)],
                8: [lambda: V(9)],
                9: [lambda: V(10)],
                10: [lambda: V(11)],
                11: [lambda: V(12)],
                12: [lambda: V(13)],
                13: [lambda: V(14)],
                14: [lambda: V(15)],
            }
            attn(0, 0, 512, f00)

            f01 = {
                0: [lambda: K(1, wkts, kth, 0)],
                2: [lambda: K(1, wkts, kth, 1)],
                4: [lambda: K(1, wkts, kth, 2)],
                6: [lambda: K(1, wkts, kth, 3)],
                8: [lambda: K(1, wqts, qth, 0)],
            }
            attn(0, 512, 512, f01)

            f10 = {
                0: [lambda: K(1, wqts, qth, 1)],
                4: [lambda: K(0, wqts, qth, 2)],
            }
            p10 = attn(1, 0, 512, f10)

            f11 = {2: [lambda: K(1, wqts, qth, 2)]}
            p11 = attn(1, 512, 512, f11)
            out_proj(rows_of(0, 512), after=p11)

            f02 = {4: [lambda: K(0, wqts, qth, 3)]}
            p02 = attn(0, 1024, 512, f02)
            out_proj(rows_of(512, 512), after=p02)

            f12 = {2: [lambda: K(1, wqts, qth, 3)]}
            p12 = attn(1, 1024, 512, f12)

            p03a = attn(0, 1536, 256)
            out_proj(rows_of(1024, 512), after=p03a)
            p13a = attn(1, 1536, 256)
            p03b = attn(0, 1792, 256)
            out_proj(rows_of(1536, 256), after=p03b)
            p13b = attn(1, 1792, 256)

            # last subchunk: hp0 contribution overlapped inside attn(1, 3b)
            oas = []
            for qt, rows in enumerate(rows_of(1792, 256)):
                oa = op_.tile([128, 1024], f32, tag=f"oa{qt}", name=f"oa{qt}",
                              bufs=1)
                oas.append(oa)
                for ec in range(2):
                    esl = slice(ec * 512, (ec + 1) * 512)
                    po = pop.tile([128, 512], f32, tag="o", name="o")
                    mm = nc.tensor.matmul(po[:], ctxt[0][:, rows],
                                          wots[:, 0, esl],
                                          start=True, stop=True)
                    pin = p13b[min(2 * (2 * qt + ec) + 1, len(p13b) - 1)]
                    add_dep_helper(mm.ins, pin.ins, sync=False,
                                   reason="last-chunk hp0 half")
                    nc.vector.tensor_copy(oa[:, esl], po[:])
            for qt, rows in enumerate(rows_of(1792, 256)):
                ot = op_.tile([128, 1024], f32, tag="ot", name="ot")
                for ec in range(2):
                    esl = slice(ec * 512, (ec + 1) * 512)
                    po = pop.tile([128, 512], f32, tag="o", name="o")
                    nc.tensor.matmul(po[:], ctxt[1][:, rows],
                                     wots[:, 1, esl], start=True, stop=True)
                    nc.vector.scalar_tensor_tensor(
                        ot[:, esl], po[:], 1.0, oas[qt][:, esl],
                        op0=mybir.AluOpType.mult, op1=mybir.AluOpType.add)
                OUT_ENG[qt % 2].dma_start(out[rows, :], ot[:])
    nc.compile()
    return nc


_CACHED = {}


def _get_nc():
    if "nc" not in _CACHED:
        _CACHED["nc"] = _build()
    return _CACHED["nc"]


def make_in_maps(x, w_qkv, w_o):
    import ml_dtypes
    bf = lambda a: np.ascontiguousarray(a).astype(ml_dtypes.bfloat16)  # noqa
    wq, wk, wv = (w_qkv[i * D:(i + 1) * D] for i in range(3))
    in_maps = []
    for c in range(NCORES):
        b, g = divmod(c, 4)
        gs = slice(g * GD, (g + 1) * GD)
        xT = x[b].T                                   # [1024, 2048]
        # [128, 4, 8, 512]: (p, quarter, d, col)
        xq = xT.reshape(ND, 128, 4, 512).transpose(1, 2, 0, 3)
        tw = lambda w: w[gs].T.reshape(ND, 128, GD).transpose(1, 0, 2)  # noqa
        wo_t = w_o[:, gs].T.reshape(2, 128, D).transpose(1, 0, 2)
        in_maps.append({
            "xq": bf(xq).reshape(128, -1),
            "wqT": bf(tw(wq)).reshape(128, -1),
            "wkT": bf(tw(wk)).reshape(128, -1),
            "wvT": bf(tw(wv)).reshape(128, -1),
            "woT": bf(wo_t).reshape(128, -1),
        })
    return in_maps


def assemble(results):
    out = np.empty((2, L, D), np.float32)
    for b in range(2):
        out[b] = sum(results[4 * b + g]["out"] for g in range(4))
    return out


def kernel(x, w_qkv, w_o):
    from concourse import bass_utils
    nc = _get_nc()
    in_maps = make_in_maps(np.asarray(x, np.float32),
                           np.asarray(w_qkv, np.float32),
                           np.asarray(w_o, np.float32))
    res = bass_utils.run_bass_kernel_spmd(
        nc, in_maps, core_ids=list(range(NCORES)))
    return assemble(res.results)


# revision 12
# speedup vs baseline: 1.1076x; 1.1076x over previous
"""Multi-head attention (B=2, L=2048, D=1024, H=16, DK=64) on 8 TRN2 NeuronCores.

Sharding: core c handles batch b = c//4 and head-group g = c%4 (4 heads,
256 model dims). Per-core compute (no collectives):
  QT/KT  [256, 2048] projections in [dk, seq] layout (rhs = x^T, lhsT = w^T)
  V      [2048, 260] with a fused ones-column per head (softmax Z for free)
  S^T    = K_h @ Q_h^T per head in [keys, queries] layout (row-packed head
           pairs run CONCURRENT on the PE array, K=64 each)
  P      = exp(S^T / 8)     one ACT op per (key-tile, head-pair)
  ctx^T  = V'_h^T @ P       -> rows 0:64 ctx, row 64 = Z
  ctx   /= Z                (reciprocal + gpsimd partition broadcast + mul)
  out_g  = ctx_g @ w_o[:, g]^T   -> per-core PARTIAL output [2048, 1024]
Host sums the 4 head-group partials per batch and stacks the 2 batches.

Schedule: ACT(exp)-centric. The scalar engine is the roofline (128 exps of
FD=1024 at (1024+352)/1.2 ns each ~= 147us); everything else (projections,
out-proj, DMA) is filler emitted around the exp stream. Unit order
(0,0),(0,1),(1,0),(1,1),(0,2),(1,2) then the last query chunk split in two
256-wide subchunks to shrink the tail. Inputs land as contiguous pre-tiled
blocks on both HW DMA queues + gpsimd SWDGE; outputs alternate HW queues.
"""

import numpy as np

D = 1024
L = 2048
DK = 64
GH = 4           # heads per core
GD = GH * DK     # model dims per core (256)
NCORES = 8
ND = D // 128    # 8 d-tiles
NL = L // 128    # 16 key tiles


def _build():
    import concourse.bacc as bacc
    import concourse.mybir as mybir
    import concourse.tile as tile
    from concourse.tile import add_dep_helper

    f32 = mybir.dt.float32
    bf16 = mybir.dt.bfloat16
    Exp = mybir.ActivationFunctionType.Exp

    nc = bacc.Bacc("TRN2", target_bir_lowering=False, debug=False,
                   num_devices=NCORES)
    xqd = nc.dram_tensor("xq", [128, 4 * ND * 512], bf16,
                         kind="ExternalInput").ap()
    wqd = nc.dram_tensor("wqT", [128, ND * GD], bf16, kind="ExternalInput").ap()
    wkd = nc.dram_tensor("wkT", [128, ND * GD], bf16, kind="ExternalInput").ap()
    wvd = nc.dram_tensor("wvT", [128, ND * GD], bf16, kind="ExternalInput").ap()
    wod = nc.dram_tensor("woT", [128, 2 * D], bf16, kind="ExternalInput").ap()
    out = nc.dram_tensor("out", [L, D], f32, kind="ExternalOutput").ap()

    with tile.TileContext(nc) as tc:
        with (
            tc.tile_pool(name="xp", bufs=1) as xp,
            tc.tile_pool(name="wp", bufs=1) as wp,
            tc.tile_pool(name="qk", bufs=1) as qk,
            tc.tile_pool(name="vp", bufs=1) as vp,
            tc.tile_pool(name="cx", bufs=1) as cx,
            tc.tile_pool(name="pp", bufs=9) as pp,
            tc.tile_pool(name="rp", bufs=4) as rp,
            tc.tile_pool(name="op", bufs=3) as op_,
            tc.tile_pool(name="ps", bufs=2, space="PSUM") as ps,
            tc.tile_pool(name="pc", bufs=2, space="PSUM") as pc,
            tc.tile_pool(name="po", bufs=2, space="PSUM") as pop,
        ):
            # ---- scratch for warmup ----------------------------------------
            wsb = wp.tile([128, 512], bf16, tag="wsb", name="wsb")
            nc.vector.memset(wsb[:], 0.0)

            # ---- DMA inputs (emitted before any scalar-engine compute so
            # the scalar HWDGE ring starts immediately) ----------------------
            wkts = wp.tile([128, ND, GD], bf16, tag="wk", name="wk")
            wqts = wp.tile([128, ND, GD], bf16, tag="wq", name="wq")
            wvts = wp.tile([128, ND, GD], bf16, tag="wv", name="wv")
            wots = wp.tile([128, 2, D], bf16, tag="wo", name="wo")
            xta = xp.tile([128, 4, ND, 512], bf16, tag="x", name="x")
            xr = xqd.rearrange("p (q d c) -> p q d c", q=4, d=ND)
            # x quarter q lands as two d-halves, one per HW queue, in
            # ascending-q order so the K/Q/V chunk chase never starves.
            nc.sync.dma_start(wkts[:], wkd.rearrange("p (d c) -> p d c", d=ND))
            nc.scalar.dma_start(wqts[:], wqd.rearrange("p (d c) -> p d c", d=ND))
            for q in range(4):
                nc.sync.dma_start(xta[:, q, 0:4], xr[:, q, 0:4])
                nc.scalar.dma_start(xta[:, q, 4:8], xr[:, q, 4:8])
            nc.gpsimd.dma_start(wvts[:], wvd.rearrange("p (d c) -> p d c", d=ND))
            nc.gpsimd.dma_start(wots[:], wod.rearrange("p (i c) -> p i c", i=2))

            # ---- engine warmup (runs during the DMA wait) -------------------
            wact = wp.tile([128, 128], bf16, tag="wact", name="wact")
            # triggers the exp ACT_TABLE_LOAD early
            nc.scalar.activation(wact[:], wsb[:, 0:128], Exp, scale=0.125)
            wpo = pop.tile([128, 512], f32, tag="o", name="o")
            for _ in range(8):   # keep PE busy so HAM un-throttles early
                nc.tensor.matmul(wpo[:], wsb[:, 0:128], wsb[:],
                                 start=True, stop=True)

            # ---- persistent SBUF tensors -----------------------------------
            # V with ones column per head: [keys 128, key-tile, DK+1]
            vph = [vp.tile([128, NL, DK + 1], bf16, tag=f"v{h}", name=f"v{h}")
                   for h in range(GH)]
            onesc = wp.tile([128, NL, 1], f32, tag="ones", name="ones")
            nc.vector.memset(onesc[:], 1.0)
            for h in range(GH):
                nc.vector.tensor_copy(vph[h][:, :, DK:DK + 1], onesc[:])
            qth = [qk.tile([128, L], bf16, tag=f"q{hp}", name=f"q{hp}")
                   for hp in range(2)]
            kth = [qk.tile([128, L], bf16, tag=f"k{hp}", name=f"k{hp}")
                   for hp in range(2)]
            ctxt = [cx.tile([128, L], bf16, tag=f"c{hp}", name=f"c{hp}")
                    for hp in range(2)]

            # ---- helpers ----------------------------------------------------
            def proj_qk_chunk(hp, w_sb, dstl, qc):
                acc = pop.tile([128, 512], f32, tag="o", name="o")
                for d in range(ND):
                    nc.tensor.matmul(
                        acc[:], w_sb[:, d, hp * 128:(hp + 1) * 128],
                        xta[:, qc, d, :],
                        start=(d == 0), stop=(d == ND - 1))
                nc.vector.tensor_copy(
                    dstl[hp][:, qc * 512:(qc + 1) * 512], acc[:])

            def v_proj_tile(lt):
                q, r = lt // 4, lt % 4
                acc = pop.tile([128, 512], f32, tag="o", name="o")
                for d in range(ND):
                    nc.tensor.matmul(
                        acc[:, 0:GD],
                        xta[:, q, d, r * 128:(r + 1) * 128],
                        wvts[:, d, :],
                        start=(d == 0), stop=(d == ND - 1))
                for h in range(GH):
                    nc.vector.tensor_copy(
                        vph[h][:, lt, 0:DK],
                        acc[:, h * DK:(h + 1) * DK])

            def attn(hp, q0, qw, fillers=None):
                """Attention for head-pair hp over queries [q0, q0+qw).

                fillers: dict lt -> list of callables, emitted after the lt's
                ctx matmuls (fills PE while ACT chews). Ends with the
                Z-division. Returns the per-lt tail ctx matmuls for pins.

                For qw=256 the two heads' score outputs go to separate PSUM
                banks (column offsets 0 and 512 of the s tile) so the
                row-packed matmul pair never shares a bank write port.
                """
                qsl = slice(q0, q0 + qw)
                cps = [pc.tile([DK + 1, 512], f32, tag="c", name=f"c{i}")
                       for i in range(2)]
                ctx_mms = []
                fd = 512 + qw   # activation covers [0 : 512+qw) of the s tile
                for lt in range(NL):
                    lsl = slice(lt * 128, (lt + 1) * 128)
                    sp = ps.tile([128, 1024], f32, tag="s", name="s")
                    for i in range(2):
                        row = slice(i * 64, (i + 1) * 64)
                        nc.tensor.matmul(
                            sp[:, i * 512:i * 512 + qw],
                            kth[hp][row, lsl], qth[hp][row, qsl],
                            start=True, stop=True,
                            tile_position=(i * 64, 0))
                    p = pp.tile([128, 1024], bf16, tag="p", name="p")
                    nc.scalar.activation(p[:, 0:fd], sp[:, 0:fd],
                                         Exp, scale=0.125)
                    for i in range(2):
                        mm = nc.tensor.matmul(
                            cps[i][:, 0:qw], vph[2 * hp + i][:, lt, :],
                            p[:, i * 512:i * 512 + qw],
                            start=(lt == 0), stop=(lt == NL - 1))
                        if i == 1:
                            ctx_mms.append(mm)
                    if fillers is not None and lt in fillers:
                        for f in fillers[lt]:
                            f()
                # division: evacuate ctx + Z, reciprocal, broadcast, multiply
                cus, rzs = [], []
                for i in range(2):
                    cu = pp.tile([64, 512], bf16, tag=f"cu{i}",
                                 name=f"cu{i}", bufs=2)
                    nc.vector.tensor_copy(cu[:, 0:qw], cps[i][0:DK, 0:qw])
                    zi = rp.tile([1, 512], f32, tag="zi", name="zi")
                    nc.vector.tensor_copy(zi[:, 0:qw], cps[i][DK:DK + 1, 0:qw])
                    cus.append(cu)
                    rzs.append(zi)
                for i in range(2):
                    rz = rp.tile([1, 512], f32, tag="rz", name="rz")
                    nc.vector.reciprocal_approx_fast(rz[:, 0:qw],
                                                     rzs[i][:, 0:qw])
                    rzs[i] = rz
                for i in range(2):
                    rzb = rp.tile([64, 512], f32, tag="rzb", name="rzb")
                    nc.gpsimd.partition_broadcast(rzb[:, 0:qw], rzs[i][:, 0:qw])
                    nc.vector.tensor_mul(
                        ctxt[hp][i * 64:(i + 1) * 64, qsl],
                        cus[i][:, 0:qw], rzb[:, 0:qw])
                return ctx_mms

            def out_proj(rows128, after=None, unit0=0):
                """Full output projection for the given 128-query row tiles.

                Stores go on the sync queue only: a dma_start costs ~1.2us of
                issuing-engine time, which must not come out of the scalar
                engine's exp budget.
                """
                unit = unit0
                for qt, rows in enumerate(rows128):
                    ot = op_.tile([128, 1024], f32, tag="ot", name="ot")
                    for ec in range(2):
                        esl = slice(ec * 512, (ec + 1) * 512)
                        po = pop.tile([128, 512], f32, tag="o", name="o")
                        for hp in range(2):
                            mm = nc.tensor.matmul(
                                po[:], ctxt[hp][:, rows],
                                wots[:, hp, esl],
                                start=(hp == 0), stop=(hp == 1))
                            if hp == 0 and after is not None:
                                pin = after[min(2 * unit + 1, len(after) - 1)]
                                add_dep_helper(mm.ins, pin.ins, sync=False,
                                               reason="pipeline out_proj")
                        unit += 1
                        nc.vector.tensor_copy(ot[:, esl], po[:])
                    nc.sync.dma_start(out[rows, :], ot[:])

            def rows_of(q0, qw):
                return [slice(q0 + qt * 128, q0 + (qt + 1) * 128)
                        for qt in range(qw // 128)]

            # ---- schedule ---------------------------------------------------
            proj_qk_chunk(0, wkts, kth, 0)
            proj_qk_chunk(0, wqts, qth, 0)
            v_proj_tile(0)

            K = proj_qk_chunk
            V = v_proj_tile
            # V(j) must be emitted at a filler slot <= j-1 (before its ctx
            # matmul); the PE over-commit here drains via the deep p pool.
            # K-chunk filler slots trail the x-quarter DMA landings so a
            # DMA-blocked matmul never parks at the head of the PE queue.
            f00 = {
                0: [lambda: V(1)],
                1: [lambda: V(2)],
                2: [lambda: K(0, wkts, kth, 1), lambda: V(3)],
                3: [lambda: V(4)],
                4: [lambda: V(5)],
                5: [lambda: K(0, wkts, kth, 2), lambda: V(6)],
                6: [lambda: V(7)],
                7: [lambda: V(8)],
                8: [lambda: K(0, wkts, kth, 3), lambda: V(9)],
                9: [lambda: V(10)],
                10: [lambda: V(11)],
                11: [lambda: K(0, wqts, qth, 1), lambda: V(12)],
                12: [lambda: V(13)],
                13: [lambda: V(14)],
                14: [lambda: V(15)],
            }
            attn(0, 0, 512, f00)

            f01 = {
                0: [lambda: K(1, wkts, kth, 0)],
                2: [lambda: K(1, wkts, kth, 1)],
                4: [lambda: K(1, wkts, kth, 2)],
                6: [lambda: K(1, wkts, kth, 3)],
                8: [lambda: K(1, wqts, qth, 0)],
            }
            attn(0, 512, 512, f01)

            f10 = {
                0: [lambda: K(1, wqts, qth, 1)],
                4: [lambda: K(0, wqts, qth, 2)],
            }
            p10 = attn(1, 0, 512, f10)

            f11 = {2: [lambda: K(1, wqts, qth, 2)]}
            p11 = attn(1, 512, 512, f11)
            out_proj(rows_of(0, 512), after=p11)

            f02 = {4: [lambda: K(0, wqts, qth, 3)]}
            p02 = attn(0, 1024, 512, f02)
            out_proj(rows_of(512, 512), after=p02)

            f12 = {2: [lambda: K(1, wqts, qth, 3)]}
            attn(1, 1024, 512, f12)

            p03 = attn(0, 1536, 512)
            out_proj(rows_of(1024, 512), after=p03)
            p13 = attn(1, 1536, 512)

            # last chunk: hp0 contribution overlapped inside attn(1, 3)
            oas = []
            for qt, rows in enumerate(rows_of(1536, 512)):
                oa = op_.tile([128, 1024], f32, tag=f"oa{qt}", name=f"oa{qt}",
                              bufs=1)
                oas.append(oa)
                for ec in range(2):
                    esl = slice(ec * 512, (ec + 1) * 512)
                    po = pop.tile([128, 512], f32, tag="o", name="o")
                    mm = nc.tensor.matmul(po[:], ctxt[0][:, rows],
                                          wots[:, 0, esl],
                                          start=True, stop=True)
                    pin = p13[min(2 * (2 * qt + ec) + 1, len(p13) - 1)]
                    add_dep_helper(mm.ins, pin.ins, sync=False,
                                   reason="last-chunk hp0 half")
                    nc.vector.tensor_copy(oa[:, esl], po[:])
            # after attn(1, 3)'s division: hp1 half + add + store, with the
            # final stores spread over both HW queues (scalar's exps are done)
            for qt, rows in enumerate(rows_of(1536, 512)):
                ot = op_.tile([128, 1024], f32, tag="ot", name="ot")
                for ec in range(2):
                    esl = slice(ec * 512, (ec + 1) * 512)
                    po = pop.tile([128, 512], f32, tag="o", name="o")
                    nc.tensor.matmul(po[:], ctxt[1][:, rows],
                                     wots[:, 1, esl], start=True, stop=True)
                    nc.vector.scalar_tensor_tensor(
                        ot[:, esl], po[:], 1.0, oas[qt][:, esl],
                        op0=mybir.AluOpType.mult, op1=mybir.AluOpType.add)
                [nc.sync, nc.scalar][qt % 2].dma_start(out[rows, :], ot[:])
    nc.compile()
    return nc


_CACHED = {}


def _get_nc():
    if "nc" not in _CACHED:
        _CACHED["nc"] = _build()
    return _CACHED["nc"]


def make_in_maps(x, w_qkv, w_o):
    import ml_dtypes
    bf = lambda a: np.ascontiguousarray(a).astype(ml_dtypes.bfloat16)  # noqa
    wq, wk, wv = (w_qkv[i * D:(i + 1) * D] for i in range(3))
    in_maps = []
    for c in range(NCORES):
        b, g = divmod(c, 4)
        gs = slice(g * GD, (g + 1) * GD)
        xT = x[b].T                                   # [1024, 2048]
        # [128, 4, 8, 512]: (p, quarter, d, col)
        xq = xT.reshape(ND, 128, 4, 512).transpose(1, 2, 0, 3)
        tw = lambda w: w[gs].T.reshape(ND, 128, GD).transpose(1, 0, 2)  # noqa
        wo_t = w_o[:, gs].T.reshape(2, 128, D).transpose(1, 0, 2)
        in_maps.append({
            "xq": bf(xq).reshape(128, -1),
            "wqT": bf(tw(wq)).reshape(128, -1),
            "wkT": bf(tw(wk)).reshape(128, -1),
            "wvT": bf(tw(wv)).reshape(128, -1),
            "woT": bf(wo_t).reshape(128, -1),
        })
    return in_maps


def assemble(results):
    out = np.empty((2, L, D), np.float32)
    for b in range(2):
        out[b] = sum(results[4 * b + g]["out"] for g in range(4))
    return out


def kernel(x, w_qkv, w_o):
    from concourse import bass_utils
    nc = _get_nc()
    in_maps = make_in_maps(np.asarray(x, np.float32),
                           np.asarray(w_qkv, np.float32),
                           np.asarray(w_o, np.float32))
    res = bass_utils.run_bass_kernel_spmd(
        nc, in_maps, core_ids=list(range(NCORES)))
    return assemble(res.results)


# revision 19
# speedup vs baseline: 1.1166x; 1.0082x over previous
"""Multi-head attention (B=2, L=2048, D=1024, H=16, DK=64) on 8 TRN2 NeuronCores.

Sharding: core c handles batch b = c//4 and head-group g = c%4 (4 heads,
256 model dims). Per-core compute (no collectives):
  QT/KT  [256, 2048] projections in [dk, seq] layout (rhs = x^T, lhsT = w^T)
  V      [2048, 260] with a fused ones-column per head (softmax Z for free)
  S^T    = K_h @ Q_h^T per head in [keys, queries] layout (row-packed head
           pairs run CONCURRENT on the PE array, K=64 each)
  P      = exp(S^T / 8)     one ACT op per (key-tile, head-pair)
  ctx^T  = V'_h^T @ P       -> rows 0:64 ctx, row 64 = Z
  ctx   /= Z                (reciprocal + gpsimd partition broadcast + mul)
  out_g  = ctx_g @ w_o[:, g]^T   -> per-core PARTIAL output [2048, 1024]
Host sums the 4 head-group partials per batch and stacks the 2 batches.

Schedule: ACT(exp)-centric. The scalar engine is the roofline (128 exps of
FD=1024 at (1024+352)/1.2 ns each ~= 147us); everything else (projections,
out-proj, DMA) is filler emitted around the exp stream. Unit order
(0,0),(0,1),(1,0),(1,1),(0,2),(1,2) then the last query chunk split in two
256-wide subchunks to shrink the tail. Inputs land as contiguous pre-tiled
blocks on both HW DMA queues + gpsimd SWDGE; outputs alternate HW queues.
"""

import numpy as np

D = 1024
L = 2048
DK = 64
GH = 4           # heads per core
GD = GH * DK     # model dims per core (256)
NCORES = 8
ND = D // 128    # 8 d-tiles
NL = L // 128    # 16 key tiles


def _build():
    import concourse.bacc as bacc
    import concourse.mybir as mybir
    import concourse.tile as tile
    from concourse.tile import add_dep_helper

    f32 = mybir.dt.float32
    bf16 = mybir.dt.bfloat16
    Exp = mybir.ActivationFunctionType.Exp

    nc = bacc.Bacc("TRN2", target_bir_lowering=False, debug=False,
                   num_devices=NCORES)
    xqd = nc.dram_tensor("xq", [128, 4 * ND * 512], bf16,
                         kind="ExternalInput").ap()
    wqd = nc.dram_tensor("wqT", [128, ND * GD], bf16, kind="ExternalInput").ap()
    wkd = nc.dram_tensor("wkT", [128, ND * GD], bf16, kind="ExternalInput").ap()
    wvd = nc.dram_tensor("wvT", [128, ND * GD], bf16, kind="ExternalInput").ap()
    wod = nc.dram_tensor("woT", [128, 2 * D], bf16, kind="ExternalInput").ap()
    out = nc.dram_tensor("out", [L, D], f32, kind="ExternalOutput").ap()

    with tile.TileContext(nc) as tc:
        with (
            tc.tile_pool(name="xp", bufs=1) as xp,
            tc.tile_pool(name="wp", bufs=1) as wp,
            tc.tile_pool(name="qk", bufs=1) as qk,
            tc.tile_pool(name="vp", bufs=1) as vp,
            tc.tile_pool(name="cx", bufs=1) as cx,
            tc.tile_pool(name="pp", bufs=9) as pp,
            tc.tile_pool(name="rp", bufs=4) as rp,
            tc.tile_pool(name="op", bufs=3) as op_,
            tc.tile_pool(name="ps", bufs=2, space="PSUM") as ps,
            tc.tile_pool(name="pc", bufs=2, space="PSUM") as pc,
            tc.tile_pool(name="po", bufs=2, space="PSUM") as pop,
        ):
            # ---- scratch for warmup ----------------------------------------
            wsb = wp.tile([128, 512], bf16, tag="wsb", name="wsb")
            nc.vector.memset(wsb[:], 0.0)

            # ---- DMA inputs (emitted before any scalar-engine compute so
            # the scalar HWDGE ring starts immediately) ----------------------
            wkts = wp.tile([128, ND, GD], bf16, tag="wk", name="wk")
            wqts = wp.tile([128, ND, GD], bf16, tag="wq", name="wq")
            wvts = wp.tile([128, ND, GD], bf16, tag="wv", name="wv")
            wots = wp.tile([128, 2, D], bf16, tag="wo", name="wo")
            xta = xp.tile([128, 4, ND, 512], bf16, tag="x", name="x")
            xr = xqd.rearrange("p (q d c) -> p q d c", q=4, d=ND)
            wkr = wkd.rearrange("p (d c) -> p d c", d=ND)
            wqr = wqd.rearrange("p (d c) -> p d c", d=ND)
            # Everything lands as d-halves, one per HW queue, in the order
            # the compute chase needs it: wk, wq, then x quarters ascending.
            nc.sync.dma_start(wkts[:, 0:4], wkr[:, 0:4])
            nc.scalar.dma_start(wkts[:, 4:8], wkr[:, 4:8])
            nc.sync.dma_start(wqts[:, 0:4], wqr[:, 0:4])
            nc.scalar.dma_start(wqts[:, 4:8], wqr[:, 4:8])
            for q in range(4):
                nc.sync.dma_start(xta[:, q, 0:4], xr[:, q, 0:4])
                nc.scalar.dma_start(xta[:, q, 4:8], xr[:, q, 4:8])
            nc.gpsimd.dma_start(wvts[:], wvd.rearrange("p (d c) -> p d c", d=ND))
            nc.gpsimd.dma_start(wots[:], wod.rearrange("p (i c) -> p i c", i=2))

            # ---- engine warmup (runs during the DMA wait) -------------------
            wact = wp.tile([128, 128], bf16, tag="wact", name="wact")
            # triggers the exp ACT_TABLE_LOAD early
            nc.scalar.activation(wact[:], wsb[:, 0:128], Exp, scale=0.125)
            wpo = pop.tile([128, 512], f32, tag="o", name="o")
            for _ in range(8):   # keep PE busy so HAM un-throttles early
                nc.tensor.matmul(wpo[:], wsb[:, 0:128], wsb[:],
                                 start=True, stop=True)

            # ---- persistent SBUF tensors -----------------------------------
            # V with ones column per head: [keys 128, key-tile, DK+1]
            vph = [vp.tile([128, NL, DK + 1], bf16, tag=f"v{h}", name=f"v{h}")
                   for h in range(GH)]
            onesc = wp.tile([128, NL, 1], f32, tag="ones", name="ones")
            nc.vector.memset(onesc[:], 1.0)
            for h in range(GH):
                nc.vector.tensor_copy(vph[h][:, :, DK:DK + 1], onesc[:])
            qth = [qk.tile([128, L], bf16, tag=f"q{hp}", name=f"q{hp}")
                   for hp in range(2)]
            kth = [qk.tile([128, L], bf16, tag=f"k{hp}", name=f"k{hp}")
                   for hp in range(2)]
            ctxt = [cx.tile([128, L], bf16, tag=f"c{hp}", name=f"c{hp}")
                    for hp in range(2)]

            # ---- helpers ----------------------------------------------------
            def proj_qk_chunk(hp, w_sb, dstl, qc):
                acc = pop.tile([128, 512], f32, tag="o", name="o")
                for d in range(ND):
                    nc.tensor.matmul(
                        acc[:], w_sb[:, d, hp * 128:(hp + 1) * 128],
                        xta[:, qc, d, :],
                        start=(d == 0), stop=(d == ND - 1))
                nc.vector.tensor_copy(
                    dstl[hp][:, qc * 512:(qc + 1) * 512], acc[:])

            def v_proj_tile(lt):
                q, r = lt // 4, lt % 4
                acc = pop.tile([128, 512], f32, tag="o", name="o")
                for d in range(ND):
                    nc.tensor.matmul(
                        acc[:, 0:GD],
                        xta[:, q, d, r * 128:(r + 1) * 128],
                        wvts[:, d, :],
                        start=(d == 0), stop=(d == ND - 1))
                for h in range(GH):
                    nc.vector.tensor_copy(
                        vph[h][:, lt, 0:DK],
                        acc[:, h * DK:(h + 1) * DK])

            def attn(hp, q0, qw, fillers=None, last=False):
                """Attention for head-pair hp over queries [q0, q0+qw).

                fillers: dict lt -> list of callables, emitted after the lt's
                ctx matmuls (fills PE while ACT chews). Ends with the
                Z-division. Returns the per-lt tail ctx matmuls for pins.

                For qw=256 the two heads' score outputs go to separate PSUM
                banks (column offsets 0 and 512 of the s tile) so the
                row-packed matmul pair never shares a bank write port.
                """
                qsl = slice(q0, q0 + qw)
                cps = [pc.tile([DK + 1, 512], f32, tag="c", name=f"c{i}")
                       for i in range(2)]
                ctx_mms = []
                fd = 512 + qw   # activation covers [0 : 512+qw) of the s tile
                for lt in range(NL):
                    lsl = slice(lt * 128, (lt + 1) * 128)
                    sp = ps.tile([128, 1024], f32, tag="s", name="s")
                    for i in range(2):
                        row = slice(i * 64, (i + 1) * 64)
                        nc.tensor.matmul(
                            sp[:, i * 512:i * 512 + qw],
                            kth[hp][row, lsl], qth[hp][row, qsl],
                            start=True, stop=True,
                            tile_position=(i * 64, 0))
                    p = pp.tile([128, 1024], bf16, tag="p", name="p")
                    nc.scalar.activation(p[:, 0:fd], sp[:, 0:fd],
                                         Exp, scale=0.125)
                    for i in range(2):
                        mm = nc.tensor.matmul(
                            cps[i][:, 0:qw], vph[2 * hp + i][:, lt, :],
                            p[:, i * 512:i * 512 + qw],
                            start=(lt == 0), stop=(lt == NL - 1))
                        if i == 1:
                            ctx_mms.append(mm)
                    if fillers is not None and lt in fillers:
                        for f in fillers[lt]:
                            f()
                # division: reciprocal chain first (its latency gates the
                # muls), then the bulk ctx evacuation, then broadcast+mul.
                # In the last unit the scalar engine is out of exp work, so
                # the evacuation copies move there.
                rzs = []
                for i in range(2):
                    zi = rp.tile([1, 512], f32, tag="zi", name="zi")
                    nc.vector.tensor_copy(zi[:, 0:qw], cps[i][DK:DK + 1, 0:qw])
                    rz = rp.tile([1, 512], f32, tag="rz", name="rz")
                    nc.vector.reciprocal_approx_fast(rz[:, 0:qw], zi[:, 0:qw])
                    rzs.append(rz)
                cus = []
                for i in range(2):
                    cu = pp.tile([64, 512], bf16, tag=f"cu{i}",
                                 name=f"cu{i}", bufs=2)
                    if last:
                        nc.scalar.copy(cu[:, 0:qw], cps[i][0:DK, 0:qw])
                    else:
                        nc.vector.tensor_copy(cu[:, 0:qw], cps[i][0:DK, 0:qw])
                    cus.append(cu)
                for i in range(2):
                    rzb = rp.tile([64, 512], f32, tag="rzb", name="rzb")
                    nc.gpsimd.partition_broadcast(rzb[:, 0:qw], rzs[i][:, 0:qw])
                    nc.vector.tensor_mul(
                        ctxt[hp][i * 64:(i + 1) * 64, qsl],
                        cus[i][:, 0:qw], rzb[:, 0:qw])
                return ctx_mms

            def out_proj(rows128, after=None, unit0=0):
                """Full output projection for the given 128-query row tiles.

                Stores go on the sync queue only: a dma_start costs ~1.2us of
                issuing-engine time, which must not come out of the scalar
                engine's exp budget.
                """
                unit = unit0
                for qt, rows in enumerate(rows128):
                    ot = op_.tile([128, 1024], f32, tag="ot", name="ot")
                    for ec in range(2):
                        esl = slice(ec * 512, (ec + 1) * 512)
                        po = pop.tile([128, 512], f32, tag="o", name="o")
                        for hp in range(2):
                            mm = nc.tensor.matmul(
                                po[:], ctxt[hp][:, rows],
                                wots[:, hp, esl],
                                start=(hp == 0), stop=(hp == 1))
                            if hp == 0 and after is not None:
                                pin = after[min(unit, len(after) - 1)]
                                add_dep_helper(mm.ins, pin.ins, sync=False,
                                               reason="pipeline out_proj")
                        unit += 1
                        nc.vector.tensor_copy(ot[:, esl], po[:])
                    nc.sync.dma_start(out[rows, :], ot[:])

            def rows_of(q0, qw):
                return [slice(q0 + qt * 128, q0 + (qt + 1) * 128)
                        for qt in range(qw // 128)]

            # ---- schedule ---------------------------------------------------
            proj_qk_chunk(0, wkts, kth, 0)
            proj_qk_chunk(0, wqts, qth, 0)
            v_proj_tile(0)

            K = proj_qk_chunk
            V = v_proj_tile
            # V(j) must be emitted at a filler slot <= j-1 (before its ctx
            # matmul); the PE over-commit here drains via the deep p pool.
            # K-chunk filler slots trail the x-quarter DMA landings so a
            # DMA-blocked matmul never parks at the head of the PE queue.
            f00 = {
                0: [lambda: V(1)],
                1: [lambda: V(2)],
                2: [lambda: V(3)],
                3: [lambda: K(0, wkts, kth, 1), lambda: V(4)],
                4: [lambda: V(5)],
                5: [lambda: V(6)],
                6: [lambda: V(7)],
                7: [lambda: K(0, wkts, kth, 2), lambda: V(8)],
                8: [lambda: V(9)],
                9: [lambda: V(10)],
                10: [lambda: V(11)],
                11: [lambda: K(0, wkts, kth, 3), lambda: V(12)],
                12: [lambda: V(13)],
                13: [lambda: K(0, wqts, qth, 1), lambda: V(14)],
                14: [lambda: V(15)],
            }
            attn(0, 0, 512, f00)

            f01 = {
                0: [lambda: K(1, wkts, kth, 0)],
                2: [lambda: K(1, wkts, kth, 1)],
                4: [lambda: K(1, wkts, kth, 2)],
                6: [lambda: K(1, wkts, kth, 3)],
                8: [lambda: K(1, wqts, qth, 0)],
            }
            attn(0, 512, 512, f01)

            f10 = {
                0: [lambda: K(1, wqts, qth, 1)],
                4: [lambda: K(0, wqts, qth, 2)],
            }
            p10 = attn(1, 0, 512, f10)

            f11 = {2: [lambda: K(1, wqts, qth, 2)]}
            p11 = attn(1, 512, 512, f11)
            out_proj(rows_of(0, 512), after=p11)

            f02 = {4: [lambda: K(0, wqts, qth, 3)]}
            p02 = attn(0, 1024, 512, f02)
            out_proj(rows_of(512, 512), after=p02)

            f12 = {2: [lambda: K(1, wqts, qth, 3)]}
            attn(1, 1024, 512, f12)

            p03 = attn(0, 1536, 512)
            out_proj(rows_of(1024, 512), after=p03)
            p13 = attn(1, 1536, 512, last=True)

            # last chunk: hp0 contribution overlapped inside attn(1, 3)
            oas = []
            for qt, rows in enumerate(rows_of(1536, 512)):
                oa = op_.tile([128, 1024], f32, tag=f"oa{qt}", name=f"oa{qt}",
                              bufs=1)
                oas.append(oa)
                for ec in range(2):
                    esl = slice(ec * 512, (ec + 1) * 512)
                    po = pop.tile([128, 512], f32, tag="o", name="o")
                    mm = nc.tensor.matmul(po[:], ctxt[0][:, rows],
                                          wots[:, 0, esl],
                                          start=True, stop=True)
                    pin = p13[min(2 * qt + ec, len(p13) - 1)]
                    add_dep_helper(mm.ins, pin.ins, sync=False,
                                   reason="last-chunk hp0 half")
                    nc.vector.tensor_copy(oa[:, esl], po[:])
            # after attn(1, 3)'s division: hp1 half + add + store, with the
            # final half-tile stores spread over both HW queues (the scalar
            # engine's exps are done by now)
            for qt, rows in enumerate(rows_of(1536, 512)):
                ot = op_.tile([128, 1024], f32, tag="ot", name="ot")
                for ec in range(2):
                    esl = slice(ec * 512, (ec + 1) * 512)
                    po = pop.tile([128, 512], f32, tag="o", name="o")
                    nc.tensor.matmul(po[:], ctxt[1][:, rows],
                                     wots[:, 1, esl], start=True, stop=True)
                    nc.vector.scalar_tensor_tensor(
                        ot[:, esl], po[:], 1.0, oas[qt][:, esl],
                        op0=mybir.AluOpType.mult, op1=mybir.AluOpType.add)
                    [nc.sync, nc.scalar][(2 * qt + ec) % 2].dma_start(
                        out[rows, esl], ot[:, esl])
    nc.compile()
    return nc


_CACHED = {}


def _get_nc():
    if "nc" not in _CACHED:
        _CACHED["nc"] = _build()
    return _CACHED["nc"]


def make_in_maps(x, w_qkv, w_o):
    import ml_dtypes
    bf = lambda a: np.ascontiguousarray(a).astype(ml_dtypes.bfloat16)  # noqa
    wq, wk, wv = (w_qkv[i * D:(i + 1) * D] for i in range(3))
    in_maps = []
    for c in range(NCORES):
        b, g = divmod(c, 4)
        gs = slice(g * GD, (g + 1) * GD)
        xT = x[b].T                                   # [1024, 2048]
        # [128, 4, 8, 512]: (p, quarter, d, col)
        xq = xT.reshape(ND, 128, 4, 512).transpose(1, 2, 0, 3)
        tw = lambda w: w[gs].T.reshape(ND, 128, GD).transpose(1, 0, 2)  # noqa
        wo_t = w_o[:, gs].T.reshape(2, 128, D).transpose(1, 0, 2)
        in_maps.append({
            "xq": bf(xq).reshape(128, -1),
            "wqT": bf(tw(wq)).reshape(128, -1),
            "wkT": bf(tw(wk)).reshape(128, -1),
            "wvT": bf(tw(wv)).reshape(128, -1),
            "woT": bf(wo_t).reshape(128, -1),
        })
    return in_maps


def assemble(results):
    out = np.empty((2, L, D), np.float32)
    for b in range(2):
        out[b] = sum(results[4 * b + g]["out"] for g in range(4))
    return out


def kernel(x, w_qkv, w_o):
    from concourse import bass_utils
    nc = _get_nc()
    in_maps = make_in_maps(np.asarray(x, np.float32),
                           np.asarray(w_qkv, np.float32),
                           np.asarray(w_o, np.float32))
    res = bass_utils.run_bass_kernel_spmd(
        nc, in_maps, core_ids=list(range(NCORES)))
    return assemble(res.results)
